# revision 48
# baseline (speedup 1.0000x reference)
"""Multi-head causal attention (B=2, L=2048, D=1024, H=16) on 8 TRN2 cores.

Sharding: data-parallel over batch (cores 0-3 -> b=0, cores 4-7 -> b=1),
tensor-parallel over heads (each core computes 4 of the 16 heads: the
matching 256-column slice of Wq/Wk/Wv and 256-row slice of Wo).  Each core
returns a partial [L, D] output-projection contribution; the host sums the
4 partials per batch and adds bo.

Design notes:
  - the host uploads x^T and all weights pre-transposed, pre-cast to bf16
    AND pre-tiled to the SBUF layout, so the kernel has no PE transposes,
    no cast copies, and every load DMA moves 4-8KB contiguous lines
  - P (=exp scores) and V stay f32r: the scalar-engine exp writes 4-byte
    output ~20% faster than bf16, and f32r moving operands >=256 wide run
    the PE at full rate
  - phase order: K^T(w0) m0 and Q^T(w3) m0 only, then attention windows
    3,2,1,0; every other projection (K/V/Q of remaining windows) streams
    into w3's ACT-bound attention as deadline-scheduled PE filler, and the
    previous window's output projection fills the later windows, so the
    PE never starves and the kernel tail is the smallest window
  - attention inner loop is software-pipelined one k-tile ahead (S(kt+1)
    issues before PV(kt)) so exp latency never stalls the PE
  - causal diagonal blocks restrict matmul/exp ranges to the valid part
    and affine_select only touches the 128-wide diagonal strip
  - softmax normalize: denominator rows hop to partitions 0/1, one
    reciprocal_approx_fast (5x faster than the bit-exact reciprocal),
    then a [2,128]-pattern matmul broadcasts both reciprocal rows across
    128 PSUM partitions (no slow gpsimd partition_broadcast), and two DVE
    multiplies scale O'^T; deferred one window to keep engine queues clear
  - last-window tail: per-head-pair normalize chains with their own
    odd-row hops, and the output projection is split into m0 (head pair
    0, issued during the normalize chain) and m1 accumulation phases with
    PSUM evacuations alternating between DVE and the scalar engine
"""

import numpy as np

import concourse.bass as bass
import concourse.tile as tile
from concourse import bacc, mybir
from concourse.bass_utils import run_bass_kernel_spmd
from concourse.vector_clock import VectorClock, ScopedClock

F32 = mybir.dt.float32
F32R = mybir.dt.float32r
BF16 = mybir.dt.bfloat16

B, L, D, H = 2, 2048, 1024, 16
DKH = 64          # head dim
HC = 4            # heads per core
DKC = HC * DKH    # 256 projected cols per core
LW = 512          # query window
NW = L // LW      # 4 windows
NKT = L // 128    # 16 k tiles


class _SplitDrainTileContext(tile.TileContext):
    """The walrus build in this container only supports a single sync-wait
    per Drain instruction; split the kernel-tail drain into one drain per
    outstanding semaphore."""

    def _drain_and_barrier(self, tick_clock, wait_clock):
        gc = tick_clock.global_clock
        n = len(gc)
        active = [i for i in range(n) if gc[i] > 0]
        for i in active:
            vc = VectorClock([gc[j] if j == i else 0 for j in range(n)])
            di = self.nc.sync.drain()
            wait_clock.add_sem_waits(di.ins, ScopedClock({None: vc}))
        self.nc.all_engine_barrier()
        popped = self.nc._tile_sem_poison_stack.pop()
        assert popped is self._sem_poison
        self.nc.clear_and_free_semaphores(list(self.sems.allocated().values()))
        self.nc.all_engine_barrier()


def _ms(ap):
    """memset-safe view: walrus rejects f32r memsets."""
    return ap.bitcast(F32) if ap.dtype == F32R else ap


def build_program() -> bass.Bass:
    nc = bacc.Bacc("TRN2", target_bir_lowering=False, debug=False)

    # x^T streams and weights arrive pre-transposed, pre-cast AND pre-tiled
    # on the host so every DMA line is 4-8KB contiguous (1KB lines only
    # reach ~290 GB/s; these reach ~435)
    xt_ctx = nc.declare_dram_parameter("xt_ctx", [NW, 128, 8, LW], BF16, isOutput=False)
    xt_val = nc.declare_dram_parameter("xt_val", [NW, 128, 8, LW], BF16, isOutput=False)
    wq = nc.declare_dram_parameter("wq", [128, 8, DKC], BF16, isOutput=False)
    wk = nc.declare_dram_parameter("wk", [128, 8, DKC], BF16, isOutput=False)
    wv = nc.declare_dram_parameter("wv", [128, 8, DKC], BF16, isOutput=False)
    bq = nc.declare_dram_parameter("bq", [DKC], F32, isOutput=False)
    bk = nc.declare_dram_parameter("bk", [DKC], F32, isOutput=False)
    bv = nc.declare_dram_parameter("bv", [DKC], F32, isOutput=False)
    wo = nc.declare_dram_parameter("wo", [128, 2, D], BF16, isOutput=False)
    pat2_in = nc.declare_dram_parameter("pat2", [2, 128], BF16, isOutput=False)
    out = nc.declare_dram_parameter("out", [L, D], BF16, isOutput=True)

    with _SplitDrainTileContext(nc) as tc:
        with (
            tc.tile_pool(name="consts", bufs=1) as consts,
            tc.tile_pool(name="resident", bufs=1) as resident,
            tc.tile_pool(name="pp", bufs=3) as p_pool,
            tc.tile_pool(name="norm", bufs=2) as norm_pool,
            tc.tile_pool(name="evac", bufs=4) as evac_pool,
            tc.tile_pool(name="ps_mm", bufs=2, space="PSUM") as ps_mm,
            tc.tile_pool(name="ps_s", bufs=2, space="PSUM") as ps_s,
            tc.tile_pool(name="ps_o", bufs=2, space="PSUM") as ps_o,
        ):
            # dummy activation: pulls the ~2.7us exp table load off the
            # first attention chunk's critical path
            warm = consts.tile([1, 8], F32, tag="warm")
            nc.vector.memset(warm[:, :], 0.0)
            nc.scalar.activation(warm[:, :], warm[:, :],
                                 func=mybir.ActivationFunctionType.Exp)
            # [2,128] 0/1 pattern: matmul(pat2, r[2,512]) broadcasts r row 0
            # to PSUM partitions 0..63 and row 1 to partitions 64..127.
            # Loaded from DRAM: engines can't write at partition offset 1.
            pat2 = consts.tile([2, 128], BF16, tag="pat2")
            nc.sync.dma_start(out=pat2[:, :], in_=pat2_in[:, :])
            # ---- weights + per-window x^T loads, ordered so K(w0) can
            # start after ~1MB ----
            wk_sb = consts.tile([128, 8, DKC], BF16, tag="wk")
            nc.sync.dma_start(out=wk_sb[:, :, :], in_=wk[:, :, :])
            bk_sb = consts.tile([128, 2], F32, tag="bk")
            nc.sync.dma_start(out=bk_sb[:, :], in_=bk[:].rearrange("(m p) -> p m", p=128))

            xc_sb = resident.tile([128, 8, L], BF16, tag="xc")
            xv_sb = resident.tile([128, 8, L], BF16, tag="xv")

            def load_x(dst, src, lw_, split=1):
                lsl_ = slice(lw_ * LW, (lw_ + 1) * LW)
                for h in range(split):
                    ks = slice(h * 8 // split, (h + 1) * 8 // split)
                    nc.sync.dma_start(
                        out=dst[:, ks, lsl_],
                        in_=src[lw_, :, ks, :])

            load_x(xc_sb, xt_ctx, 0, split=2)

            wq_sb = consts.tile([128, 8, DKC], BF16, tag="wq")
            nc.sync.dma_start(out=wq_sb[:, :, :], in_=wq[:, :, :])
            bq_sb = consts.tile([128, 2], F32, tag="bq")
            nc.sync.dma_start(out=bq_sb[:, :], in_=bq[:].rearrange("(m p) -> p m", p=128))

            load_x(xc_sb, xt_ctx, NW - 1)  # Q(w3) is the second projection

            wv_sb = consts.tile([128, 8, DKC], BF16, tag="wv")
            nc.sync.dma_start(out=wv_sb[:, :, :], in_=wv[:, :, :])
            # bv replicated to all 128 partitions so the V-projection bias
            # folds into the PSUM->SBUF copy on the DVE
            bv_bc = consts.tile([128, DKC], F32, tag="bv")
            nc.sync.dma_start(
                out=bv_bc[:, :], in_=bv[:].unsqueeze(0).broadcast_to([128, DKC]))

            load_x(xv_sb, xt_val, 0)
            for lw_ in range(1, NW):
                if lw_ < NW - 1:
                    load_x(xc_sb, xt_ctx, lw_)
                load_x(xv_sb, xt_val, lw_)

            wo_sb = consts.tile([128, 2, D], BF16, tag="wo")
            nc.sync.dma_start(out=wo_sb[:, :, :], in_=wo[:, :, :])

            # ---- resident projections ----
            qT_sb = resident.tile([128, 2, L], BF16, tag="qT")      # Q^T, head h at [(h%2)*64:+64, h//2, :]
            kT_sb = resident.tile([128, 2, L], BF16, tag="kT")      # K^T, same layout
            v_sb = resident.tile([128, NKT, HC, 1 + DKH], F32R, tag="v")  # V per l-tile/head: [V | ones]
            oT_sb = resident.tile([128, 2, L], BF16, tag="oT")      # normalized O^T, same layout as kT

            nc.vector.memset(_ms(v_sb[:, :, :, DKH:DKH + 1]), 1.0)  # ones col -> denominator row

            def proj_qk(dst, w_sb, b_sb, lw_, m):
                lsl_ = slice(lw_ * LW, (lw_ + 1) * LW)
                pq = ps_mm.tile([128, LW], F32, tag="mm512")
                for k in range(8):
                    nc.tensor.matmul(
                        pq[:, :],
                        w_sb[:, k, m * 128:(m + 1) * 128],
                        xc_sb[:, k, lsl_],
                        start=(k == 0), stop=(k == 7),
                    )
                nc.vector.tensor_scalar_add(dst[:, m, lsl_], pq[:, :], b_sb[:, m:m + 1])

            def proj_v(lw_, a):
                pv = ps_mm.tile([128, LW], F32, tag="mm512")
                for k in range(8):
                    nc.tensor.matmul(
                        pv[:, 0:DKC],
                        xv_sb[:, k, lw_ * LW + a * 128:lw_ * LW + (a + 1) * 128],
                        wv_sb[:, k, :],
                        start=(k == 0), stop=(k == 7),
                    )
                nc.vector.tensor_add(
                    v_sb[:, lw_ * 4 + a, :, 0:DKH],
                    pv[:, 0:DKC].rearrange("p (h d) -> p h d", h=HC),
                    bv_bc[:, :].rearrange("p (h d) -> p h d", h=HC),
                )

            # ---- Phase A: every projection that depends only on the first
            # two x windows loaded (xc0, xc3) — K(w0)/Q(w3) to unlock
            # attention hp0, then K(w0)m1/Q(w3)m1/Q(w0)/K(w3) to keep the
            # PE fed while the remaining ~3MB of x streams in.  The rest is
            # deadline-scheduled filler inside w3's attention.
            proj_qk(kT_sb, wk_sb, bk_sb, 0, 0)
            proj_qk(qT_sb, wq_sb, bq_sb, NW - 1, 0)
            proj_qk(kT_sb, wk_sb, bk_sb, 0, 1)
            proj_qk(qT_sb, wq_sb, bq_sb, NW - 1, 1)
            proj_qk(qT_sb, wq_sb, bq_sb, 0, 0)
            proj_qk(qT_sb, wq_sb, bq_sb, 0, 1)
            proj_qk(kT_sb, wk_sb, bk_sb, NW - 1, 0)
            proj_qk(kT_sb, wk_sb, bk_sb, NW - 1, 1)

            # ---- attention, windows largest-first so the tail is small ----
            pend = {}

            def finish_pair(lw_, onorm2, hp, ostg_e, ostg_o, rq):
                """Deferred half of one head pair's normalize: approx
                reciprocal, PE pattern-broadcast, DVE multiplies into O^T."""
                lsl_ = slice(lw_ * LW, (lw_ + 1) * LW)
                rqr = norm_pool.tile([2, LW], F32, tag="rqr")
                nc.vector.reciprocal_approx_fast(rqr[:, :], rq[:, :])
                rqr_b = norm_pool.tile([2, LW], BF16, tag="rqrb")
                nc.vector.tensor_copy(rqr_b[:, :], rqr[:, :])
                rb2 = ps_mm.tile([128, LW], F32, tag="mm512")
                nc.tensor.matmul(rb2[:, :], pat2[:, :], rqr_b[:, :],
                                 start=True, stop=True)
                nc.vector.tensor_mul(
                    oT_sb[0:64, hp, lsl_], ostg_e[0:DKH, :], rb2[0:64, :])
                nc.vector.tensor_mul(
                    onorm2[:, hp, :], ostg_o[0:DKH, :], rb2[64:128, :])

            def finish_normalize(lw_):
                lsl_ = slice(lw_ * LW, (lw_ + 1) * LW)
                onorm2 = norm_pool.tile([DKH, 2, LW], BF16, tag="onorm")
                for hp, ostg_e, ostg_o, rq in pend.pop(lw_):
                    finish_pair(lw_, onorm2, hp, ostg_e, ostg_o, rq)
                nc.gpsimd.dma_start(out=oT_sb[64:128, 0:2, lsl_], in_=onorm2[:, :, :])

            def last_stage2(lw_, hp_, onorm2, ostg_e_, ostg_o_, rq_):
                """Last-window normalize: the proven pat2 broadcast path,
                plus this head pair's own odd-row hop."""
                lsl_ = slice(lw_ * LW, (lw_ + 1) * LW)
                finish_pair(lw_, onorm2, hp_, ostg_e_, ostg_o_, rq_)
                nc.gpsimd.dma_start(
                    out=oT_sb[64:128, hp_:hp_ + 1, lsl_],
                    in_=onorm2[:, hp_:hp_ + 1, :])

            def out_proj_unit(lt, n):
                """One [128, 512] tile of partial = O^T.T @ Wo; the PSUM
                evacuation alternates between DVE and the scalar engine
                (copy lives in every act table, so no table reload)."""
                pop = ps_mm.tile([128, LW], F32, tag="mm512")
                for m in range(2):
                    nc.tensor.matmul(
                        pop[:, :],
                        oT_sb[:, m, lt * 128:(lt + 1) * 128],
                        wo_sb[:, m, n * 512:(n + 1) * 512],
                        start=(m == 0), stop=(m == 1),
                    )
                ost = p_pool.tile([128, LW], BF16, tag="ostage")
                if (lt + n) % 2 == 0:
                    nc.vector.tensor_copy(ost[:, :], pop[:, :])
                else:
                    nc.scalar.copy(ost[:, :], pop[:, :])
                nc.sync.dma_start(
                    out=out[lt * 128:(lt + 1) * 128, n * 512:(n + 1) * 512],
                    in_=ost[:, :],
                )

            order = list(range(NW - 1, -1, -1))  # 3, 2, 1, 0
            for wi, lw in enumerate(order):
                lsl = slice(lw * LW, (lw + 1) * LW)
                nkt = 4 * (lw + 1)
                prev = order[wi - 1] if wi > 0 else None
                if prev is not None:
                    finish_normalize(prev)
                # PE filler inside this ACT-bound window.  During w3 (the
                # first processed window) the remaining K/V/Q projections
                # stream in with deadlines: K(wa) before S(4a) is issued,
                # V(wa) unit a' before PV(4a+a') consumes it.  Later windows
                # interleave the previous window's output projection.
                if wi == 0:
                    filler = []  # (deadline, emit)
                    for a in range(4):
                        filler.append((a,
                                       lambda a_=a: proj_v(0, a_)))
                    for lw2 in range(1, NW):
                        if lw2 < NW - 1:  # K(w3) already ran in Phase A
                            for m in range(2):
                                filler.append((4 * lw2 - 1,
                                               lambda lw_=lw2, m_=m: proj_qk(kT_sb, wk_sb, bk_sb, lw_, m_)))
                        for a in range(4):
                            filler.append((4 * lw2 + a,
                                           lambda lw_=lw2, a_=a: proj_v(lw_, a_)))
                    for lw2 in range(NW - 2, 0, -1):  # Q(w0) already ran
                        for m in range(2):
                            filler.append((99,
                                           lambda lw_=lw2, m_=m: proj_qk(qT_sb, wq_sb, bq_sb, lw_, m_)))
                    gate = 0
                else:
                    filler = [(99, lambda lt_=lt, n_=n: out_proj_unit(lt_, n_))
                              for lt in range(prev * 4, prev * 4 + 4) for n in range(2)]
                    gate = max(2, nkt - 6)
                fi = 0
                last_s1 = None
                last_onorm2 = None
                for hp in range(2):  # head pairs (2hp, 2hp+1)
                    po_e = ps_o.tile([1 + DKH, LW], F32, tag="o")
                    po_o = ps_o.tile([1 + DKH, LW], F32, tag="o")
                    psbs = {}

                    def emit_S(kt):
                        s = kt - 4 * lw
                        qr = slice(128 * s, LW) if s > 0 else slice(0, LW)
                        ksb = ps_s.tile([128, 2, LW], F32, tag="s")
                        nc.tensor.matmul(
                            ksb[:, 0, qr],
                            kT_sb[0:64, hp, kt * 128:(kt + 1) * 128],
                            qT_sb[0:64, hp, lw * LW + qr.start:lw * LW + LW],
                            start=True, stop=True,
                        )
                        nc.tensor.matmul(
                            ksb[:, 1, qr],
                            kT_sb[64:128, hp, kt * 128:(kt + 1) * 128],
                            qT_sb[64:128, hp, lw * LW + qr.start:lw * LW + LW],
                            start=True, stop=True,
                        )
                        psb = p_pool.tile([128, 2, LW], F32R, tag="p")
                        nc.scalar.activation(
                            psb[:, :, qr], ksb[:, :, qr],
                            func=mybir.ActivationFunctionType.Exp,
                            scale=1.0 / np.sqrt(DKH),
                        )
                        if s >= 0:  # diagonal block: zero q<k in its 128-col strip
                            qs = slice(128 * s, 128 * s + 128)
                            for i in range(2):
                                nc.gpsimd.affine_select(
                                    out=psb[:, i, qs], in_=psb[:, i, qs],
                                    compare_op=mybir.AluOpType.is_ge,
                                    fill=0.0, base=0,
                                    pattern=[[1, 128]], channel_multiplier=-1,
                                )
                        psbs[kt] = psb

                    emit_S(0)
                    for kt in range(nkt):
                        # flush fillers: hard deadlines (hp0 of w3) always;
                        # otherwise paced at `cap` per iteration after `gate`
                        cap = 1 if wi == 0 else 2
                        flushed = 0
                        while fi < len(filler):
                            dl = filler[fi][0]
                            if (hp == 0 and dl <= kt) or (flushed < cap and kt >= gate):
                                filler[fi][1]()
                                fi += 1
                                flushed += 1
                            else:
                                break
                        if kt + 1 < nkt:
                            emit_S(kt + 1)
                        if kt == 1 and last_s1 is not None:
                            # hp0's deferred normalize lands early in hp1's
                            # loop: its DVE stage-1 ran during S(0..2)
                            last_stage2(lw, 0, last_onorm2, *last_s1)
                            last_s1 = None
                        s = kt - 4 * lw
                        qr = slice(128 * s, LW) if s > 0 else slice(0, LW)
                        psb = psbs.pop(kt)
                        nc.tensor.matmul(
                            po_e[:, qr], v_sb[:, kt, 2 * hp, :], psb[:, 0, qr],
                            start=(kt == 0), stop=(kt == nkt - 1),
                        )
                        nc.tensor.matmul(
                            po_o[:, qr], v_sb[:, kt, 2 * hp + 1, :], psb[:, 1, qr],
                            start=(kt == 0), stop=(kt == nkt - 1),
                        )
                    # evacuate both PSUM banks right away (split across DVE
                    # and the scalar engine on the last window's tail)
                    ostg_e = evac_pool.tile([1 + DKH, LW], F32, tag="ostg")
                    nc.vector.tensor_copy(ostg_e[:, :], po_e[:, :])
                    ostg_o = evac_pool.tile([1 + DKH, LW], F32, tag="ostg")
                    if wi == NW - 1 and hp == 1:
                        nc.scalar.copy(ostg_o[:, :], po_o[:, :])
                    else:
                        nc.vector.tensor_copy(ostg_o[:, :], po_o[:, :])
                    if wi == NW - 1:
                        rq = norm_pool.tile([2, LW], F32, tag="rq")
                        nc.gpsimd.dma_start(out=rq[0:1, :], in_=ostg_e[64:65, :])
                        nc.gpsimd.dma_start(out=rq[1:2, :], in_=ostg_o[64:65, :])
                        if hp == 0:
                            last_onorm2 = norm_pool.tile([DKH, 2, LW], BF16, tag="onorm")
                            last_s1 = (ostg_e, ostg_o, rq)
                        else:
                            # tail output projection, m0/m1 split with
                            # 1024-wide moving: the m=0 (head pair 0)
                            # matmuls run during hp1's normalize chain; only
                            # the short m=1 accumulation waits on it
                            def op_pair_m0(lt):
                                t = ps_s.tile([128, 2, LW], F32, tag="s")
                                for n in range(2):
                                    nc.tensor.matmul(
                                        t[:, n, :],
                                        oT_sb[:, 0, lt * 128:(lt + 1) * 128],
                                        wo_sb[:, 0, n * 512:(n + 1) * 512],
                                        start=True, stop=False,
                                    )
                                return t

                            def op_pair_m1(lt, t):
                                for n in range(2):
                                    nc.tensor.matmul(
                                        t[:, n, :],
                                        oT_sb[:, 1, lt * 128:(lt + 1) * 128],
                                        wo_sb[:, 1, n * 512:(n + 1) * 512],
                                        start=False, stop=True,
                                    )
                                ost = p_pool.tile([128, 2, LW], BF16, tag="ostage2")
                                nc.vector.tensor_copy(ost[:, 0, :], t[:, 0, :])
                                nc.scalar.copy(ost[:, 1, :], t[:, 1, :])
                                for n in range(2):
                                    nc.sync.dma_start(
                                        out=out[lt * 128:(lt + 1) * 128, n * 512:(n + 1) * 512],
                                        in_=ost[:, n, :])

                            def op_single_m0(lt, n):
                                u = ps_mm.tile([128, LW], F32, tag="mm512")
                                nc.tensor.matmul(
                                    u[:, :],
                                    oT_sb[:, 0, lt * 128:(lt + 1) * 128],
                                    wo_sb[:, 0, n * 512:(n + 1) * 512],
                                    start=True, stop=False,
                                )
                                return u

                            def op_single_m1(lt, n, u, ev):
                                nc.tensor.matmul(
                                    u[:, :],
                                    oT_sb[:, 1, lt * 128:(lt + 1) * 128],
                                    wo_sb[:, 1, n * 512:(n + 1) * 512],
                                    start=False, stop=True,
                                )
                                ost = p_pool.tile([128, LW], BF16, tag="ostage")
                                ev(ost[:, :], u[:, :])
                                nc.sync.dma_start(
                                    out=out[lt * 128:(lt + 1) * 128, n * 512:(n + 1) * 512],
                                    in_=ost[:, :])

                            base = lw * 4
                            t0 = op_pair_m0(base + 0)
                            t1 = op_pair_m0(base + 1)
                            last_stage2(lw, 1, last_onorm2, ostg_e, ostg_o, rq)
                            # tile 2's m=0 rides ps_mm's banks so the PE has
                            # work while m1(t0/t1) wait on the odd-row hop
                            u20 = op_single_m0(base + 2, 0)
                            u21 = op_single_m0(base + 2, 1)
                            op_pair_m1(base + 0, t0)
                            op_pair_m1(base + 1, t1)
                            op_single_m1(base + 2, 0, u20, nc.vector.tensor_copy)
                            op_single_m1(base + 2, 1, u21, nc.scalar.copy)
                            t3 = op_pair_m0(base + 3)
                            op_pair_m1(base + 3, t3)
                    else:
                        # denominator rows hop onto partitions 0/1 of one
                        # tile; reciprocal + multiplies are deferred a window
                        # so they never precede the next window's PSUM->SBUF
                        # copies in the DVE's in-order stream
                        rq = norm_pool.tile([2, LW], F32, tag="rq")
                        nc.gpsimd.dma_start(out=rq[0:1, :], in_=ostg_e[64:65, :])
                        nc.gpsimd.dma_start(out=rq[1:2, :], in_=ostg_o[64:65, :])
                        pend.setdefault(lw, []).append((hp, ostg_e, ostg_o, rq))
                while fi < len(filler):
                    filler[fi][1]()
                    fi += 1

    nc.compile()
    return nc


_CACHE = {}


def _program() -> bass.Bass:
    if "nc" not in _CACHE:
        _CACHE["nc"] = build_program()
    return _CACHE["nc"]


def make_in_maps(inputs):
    import ml_dtypes
    bf16 = ml_dtypes.bfloat16
    ctx = np.asarray(inputs["context_sequence"], np.float32)
    val = np.asarray(inputs["value_sequence"], np.float32)
    Wq = np.asarray(inputs["Wq"], np.float32)
    Wk = np.asarray(inputs["Wk"], np.float32)
    Wv = np.asarray(inputs["Wv"], np.float32)
    Wo = np.asarray(inputs["Wo"], np.float32)
    bq = np.asarray(inputs["bq"], np.float32)
    bk = np.asarray(inputs["bk"], np.float32)
    bv = np.asarray(inputs["bv"], np.float32)
    pat2 = np.zeros((2, 128), np.float32)
    pat2[0, 0:64] = 1.0
    pat2[1, 64:128] = 1.0
    pat2 = pat2.astype(bf16)

    def tile_x(x):
        # [L, D] -> x^T tiled [NW, 128, 8, LW]: [w, p, k, l] = xT[128k+p, 512w+l]
        return np.ascontiguousarray(
            x.T.reshape(8, 128, NW, LW).transpose(2, 1, 0, 3)).astype(bf16)

    def tile_w(w):
        # [D, n] -> [128, 8, n]: [p, k, n] = w[128k+p, n]
        return np.ascontiguousarray(
            w.reshape(8, 128, -1).transpose(1, 0, 2)).astype(bf16)

    def tile_wo(w):
        # [DKC, D] -> [128, 2, D]: [p, m, n] = w[128m+p, n]
        return np.ascontiguousarray(
            w.reshape(2, 128, -1).transpose(1, 0, 2)).astype(bf16)

    xt_ctx = [tile_x(ctx[b]) for b in range(B)]
    xt_val = [tile_x(val[b]) for b in range(B)]
    in_maps = []
    for c in range(8):
        b, hg = divmod(c, 4)
        cols = slice(hg * DKC, (hg + 1) * DKC)
        in_maps.append({
            "xt_ctx": xt_ctx[b],
            "xt_val": xt_val[b],
            "wq": tile_w(Wq[:, cols]),
            "wk": tile_w(Wk[:, cols]),
            "wv": tile_w(Wv[:, cols]),
            "bq": np.ascontiguousarray(bq[cols]),
            "bk": np.ascontiguousarray(bk[cols]),
            "bv": np.ascontiguousarray(bv[cols]),
            "wo": tile_wo(Wo[cols, :]),
            "pat2": pat2,
        })
    return in_maps


def combine_outputs(results, bo):
    bo = np.asarray(bo, np.float32)
    outs = [np.asarray(r["out"], np.float32) for r in results]
    full = np.empty((B, L, D), np.float32)
    for b in range(B):
        acc = np.zeros((L, D), np.float64)
        for c in range(4 * b, 4 * b + 4):
            acc += outs[c]
        full[b] = (acc + bo).astype(np.float32)
    return full


def _numpy_fallback(inputs):
    """Reference semantics for a non-causal mask (the TRN kernel hardcodes
    the causal structure)."""
    ctx = np.asarray(inputs["context_sequence"], np.float32)
    val = np.asarray(inputs["value_sequence"], np.float32)
    mask = np.asarray(inputs["mask"]) != 0
    Q = (ctx @ inputs["Wq"] + inputs["bq"]).reshape(B, L, H, DKH)
    Kp = (ctx @ inputs["Wk"] + inputs["bk"]).reshape(B, L, H, DKH)
    V = (val @ inputs["Wv"] + inputs["bv"]).reshape(B, L, H, DKH)
    outs = np.zeros((B, L, D), np.float32)
    for b in range(B):
        for h in range(H):
            s = (Q[b, :, h, :] @ Kp[b, :, h, :].T) / np.sqrt(np.float32(DKH))
            s = np.where(mask, s, -np.inf)
            s = s - s.max(axis=1, keepdims=True)
            p = np.exp(s)
            p /= p.sum(axis=1, keepdims=True)
            outs[b] += (p @ V[b, :, h, :]) @ np.asarray(inputs["Wo"])[h * DKH:(h + 1) * DKH, :]
    return outs + np.asarray(inputs["bo"], np.float32)


def kernel(**inputs) -> np.ndarray:
    mask = np.asarray(inputs["mask"])
    if not np.array_equal(mask != 0, np.tril(np.ones((L, L), bool))):
        return _numpy_fallback(inputs)
    nc = _program()
    in_maps = make_in_maps(inputs)
    last_err = None
    for _attempt in range(3):
        try:
            res = run_bass_kernel_spmd(nc, in_maps, list(range(8)))
            break
        except Exception as e:  # transient NRT device wedges clear on retry
            last_err = e
    else:
        raise last_err
    return combine_outputs(res.results, inputs["bo"])


if __name__ == "__main__":
    rng = np.random.default_rng(0)
    demo = {
        "context_sequence": rng.normal(size=(B, L, D)).astype(np.float32),
        "value_sequence": rng.normal(size=(B, L, D)).astype(np.float32),
        "mask": np.tril(np.ones((L, L), np.int32)),
        **{f"W{n}": (rng.normal(size=(D, D)) / 32).astype(np.float32) for n in "qkvo"},
        **{f"b{n}": (rng.normal(size=(D,)) / 32).astype(np.float32) for n in "qkvo"},
    }
    out = kernel(**demo)
    print(out.shape, out.dtype)
